# revision 88
# baseline (speedup 1.0000x reference)
"""Linformer-style multi-head attention on 8 Trainium2 NeuronCores.

Problem (hardcoded): B=4, S=4096, C=1024, H=16, D=64, DK=256, fp32 in/out.

Sharding: core i handles (batch b = i//2, head-group g = i%2 of 8 heads).
Each core computes its 8 heads' attention and the partial output
projection out_part = head_out_g @ Wo[:, g_cols].T; the host sums the two
head-group partials per batch and adds bo.

Key algebraic restructuring vs the naive path: the Linformer projections
are reassociated as Kp = (E@x)@Wk^T and Vp = (F@x)@Wv^T, so the sequence
axis (4096) is contracted down to dim_k (256) BEFORE the channel-space
weights are applied.  That removes the full-seq K/V projections entirely
(x@Wk^T and x@Wv^T at [4096,1024]@[1024,512] each) and replaces them with
E@x / F@x at [256,4096]@[4096,1024] plus two tiny [*,1024]@[1024,*]
GEMMs — about 75us less PE time per core.  All matmul operands are bf16
(fp32 PSUM accumulation); measured end-to-end relative error ~7e-3
against the fp32 reference (gate 2e-2).

Per-core kernel:
  pass 1a: stream x (natural [s,c] layout) st-tile by st-tile; one
      matmul per (st, ct) with stationary x-tile and moving [ew|fw]
      tile accumulates EFx^T[c, 2*dk] = [E@x | F@x]^T into 8 PSUM banks
      (one per c-tile) over the full sequence; copy to SBUF bf16.
  pass 1c: Kp^T[hd,dk] = wkT-tiles x ExT-slices; Vp^T[dk,hd] =
      FxT-slices x wvT-tiles (PSUM accum over the 8 c-tiles); add E/F
      biases; build the augmented [ones | Vp^T] stationary vpa.
  pass 1b: Q^T[hd,s] per 512-seq chunk from xT (c-major) and wqT,
      kept resident in SBUF (4 MiB bf16).
  pass 2, software-pipelined over (chunk, head-pair) items (scores+exp
  run 3 items ahead of AV/normalize):
      scoresT[dk,s] = Kp-slices x Q^T   (row-packed head pairs, K=64,
          concurrent via PE row groups at partition bases 0/64)
      expT = exp(scoresT/8) on ACT      (batched [128,1024] per head row)
      per head, ONE matmul group with vpa: rows 0-63 = softmax
          denominator replicated across partitions, rows 64-127 =
          unnormalized head_out^T; then ho = av * recip_approx(denom)
          on DVE (cross-partition-base operands, probed valid on HW)
      out[s,c] = ho-slices x WoT        (accumulate 4 hd blocks; PSUM
          slots shared with the AV pool; copies on ACT)
"""

import threading

import ml_dtypes
import numpy as np

B, S, C = 4, 4096, 1024
H, D, DK = 16, 64, 256
HG = 8               # heads per core
HD = HG * D          # 512
NCORES = 8
SCH = 512            # sequence chunk (pass 1b / pass 2)
NCH = S // SCH       # 8 chunks
NST = S // 128       # 32 s-tiles over the full sequence
NCT = C // 128       # 8 c-tiles
NPT = HD // 128      # 4 hd blocks (head pairs)
NDB = DK // 128      # 2 dk blocks

BF16_NP = ml_dtypes.bfloat16

_lock = threading.Lock()
_compiled = None


def _build():
    import concourse.bacc as bacc
    import concourse.bass as bass
    import concourse.tile as tile
    from concourse import mybir

    F32 = mybir.dt.float32
    BF16 = mybir.dt.bfloat16
    EXP = mybir.ActivationFunctionType.Exp

    nc = bacc.Bacc(None, target_bir_lowering=False)

    # all inputs are host-packed partition-major so every DMA moves fat
    # contiguous per-partition lines (2-8 KiB)
    xn = nc.dram_tensor("xn", [NST // 2, 128, 2 * C], BF16, kind="ExternalInput")
    xT = nc.dram_tensor("xt", [NCH, 128, NCT * SCH], BF16, kind="ExternalInput")
    effw = nc.dram_tensor(
        "effw", [NST // 2, 128, 4 * DK], BF16, kind="ExternalInput"
    )
    wqT = nc.dram_tensor("wqt", [128, NCT * HD], BF16, kind="ExternalInput")
    wkT = nc.dram_tensor("wkt", [128, NCT * HD], BF16, kind="ExternalInput")
    wvT = nc.dram_tensor("wvt", [128, NCT * HD], BF16, kind="ExternalInput")
    eb = nc.dram_tensor("eb", [DK], F32, kind="ExternalInput")
    fb = nc.dram_tensor("fb", [DK], F32, kind="ExternalInput")
    woT = nc.dram_tensor("wot", [128, NPT * C], BF16, kind="ExternalInput")
    ones = nc.dram_tensor("ones", [128, HD], BF16, kind="ExternalInput")
    out = nc.dram_tensor("out", [S, C], BF16, kind="ExternalOutput")

    with tile.TileContext(nc) as tc:
        with (
            tc.tile_pool(name="consts", bufs=1) as consts,
            tc.tile_pool(name="mids", bufs=1) as mids,
        ):
            wq_sb = consts.tile([128, NCT, HD], BF16)
            wk_sb = consts.tile([128, NCT, HD], BF16)
            wv_sb = consts.tile([128, NCT, HD], BF16)
            eb_sb = consts.tile([128, DK], F32)
            fb_sb = consts.tile([128, NDB], F32)

            # per-ct tiles so 1c matmuls only depend on their own copy
            efx_sb = [
                mids.tile([128, 2 * DK], BF16, name=f"efxsb{ct}")
                for ct in range(NCT)
            ]
            kp_sb = mids.tile([128, NPT, DK], BF16)         # Kp^T [hd, dk]
            # Augmented Vp^T: per dk-tile, per head: 64 cols of ones then
            # 64 cols of Vp^T. A single AV matmul then yields rows 0-63 =
            # the softmax denominator replicated across 64 partitions and
            # rows 64-127 = head_out^T (unnormalized).
            vpa_sb = mids.tile([128, NDB, 2 * HD], BF16)
            # Q^T, per-chunk tiles so pass 2's first scores only depend on
            # chunk 0's copies, not all 32
            qt_sb = [
                mids.tile([128, NPT, SCH], BF16, name=f"qt{ch}")
                for ch in range(NCH)
            ]
            # xt for the deferred Q chunks (long-lived: written in 1b,
            # read by pass-2 filler matmuls)
            NDEF = 4
            xt_d = {
                ch: mids.tile([128, NCT, SCH], BF16, name=f"xtd{ch}")
                for ch in range(NCH - NDEF, NCH)
            }

            # ------- pass 1a: EFx^T = [E@x | F@x]^T, full-seq accum -------
            with (
                tc.tile_pool(name="p1a", bufs=5) as p1a,
                tc.tile_pool(name="efxps", bufs=1, space="PSUM") as efxps,
            ):
                # one PSUM tile (= one bank) per c-tile so the trailing
                # copies only depend on their own bank's stop matmul
                efx_ps = [
                    efxps.tile([128, 2 * DK], F32, name=f"efx{ct}")
                    for ct in range(NCT)
                ]
                for sp in range(NST // 2):
                    xn_t = p1a.tile([128, 2, C], BF16, name="xn")
                    ef_t = p1a.tile([128, 2, 2 * DK], BF16, name="ef")
                    if sp == 0:
                        # split the first tiles into halves so the first
                        # matmul starts as early as possible
                        for half in range(2):
                            nc.sync.dma_start(ef_t[:, half, :], effw[sp][:, half * 2 * DK:(half + 1) * 2 * DK])
                            nc.sync.dma_start(xn_t[:, half, :], xn[sp][:, half * C:(half + 1) * C])
                    else:
                        nc.sync.dma_start(ef_t, effw[sp])
                        nc.sync.dma_start(xn_t, xn[sp])
                    # consts for 1c/1b, spread so no burst stalls the stream
                    if sp == 2:
                        nc.sync.dma_start(
                            wk_sb, wkT[:].rearrange("p (ct n) -> p ct n", n=HD)
                        )
                    elif sp == 3:
                        nc.sync.dma_start(
                            wv_sb, wvT[:].rearrange("p (ct n) -> p ct n", n=HD)
                        )
                    elif sp == 5:
                        nc.sync.dma_start(
                            wq_sb, wqT[:].rearrange("p (ct n) -> p ct n", n=HD)
                        )
                    elif sp == 7:
                        eb_bc = bass.AP(
                            tensor=eb[:].tensor, offset=0, ap=[[0, 128], [1, DK]]
                        )
                        nc.sync.dma_start(eb_sb[:], eb_bc)
                        for db in range(NDB):
                            fb_col = fb[db * 128:(db + 1) * 128].rearrange(
                                "(p one) -> p one", one=1
                            )
                            nc.sync.dma_start(fb_sb[:, db:db + 1], fb_col)
                    for half in range(2):
                        st = 2 * sp + half
                        for ct in range(NCT):
                            nc.tensor.matmul(
                                efx_ps[ct],
                                xn_t[:, half, ct * 128:(ct + 1) * 128],
                                ef_t[:, half, :],
                                start=(st == 0), stop=(st == NST - 1),
                            )
                            if st == NST - 1:
                                # alternate engines: copies trail the last
                                # matmuls and gate 1c
                                if ct % 2 == 0:
                                    nc.scalar.copy(efx_sb[ct], efx_ps[ct])
                                else:
                                    nc.vector.tensor_copy(efx_sb[ct], efx_ps[ct])

            # ------- pass 1c: Kp^T / Vp^T (tiny) + pass 1b: Q^T ----------
            with (
                tc.tile_pool(name="kvps", bufs=1, space="PSUM") as kvps,
                tc.tile_pool(name="qtps", bufs=2, space="PSUM") as qtps,
                tc.tile_pool(name="p1b", bufs=2) as p1b,
            ):
                kp_ps = kvps.tile([128, NPT, DK], F32)
                vp_ps = kvps.tile([128, NDB, HD], F32)
                # ct-outer so the PE starts as soon as efx_sb[ct=0] lands;
                # kp_ps slices pt={0,1} share PSUM bank 0 and pt={2,3}
                # share bank 1 — one accumulation group per bank.
                for ct in range(NCT):
                    for pt in range(NPT):
                        nc.tensor.matmul(
                            kp_ps[:, pt, :],
                            wk_sb[:, ct, pt * 128:(pt + 1) * 128],
                            efx_sb[ct][:, 0:DK],
                            start=(pt % 2 == 0 and ct == 0),
                            stop=(pt % 2 == 1 and ct == NCT - 1),
                        )
                    for db in range(NDB):
                        nc.tensor.matmul(
                            vp_ps[:, db, :],
                            efx_sb[ct][:, DK + db * 128:DK + (db + 1) * 128],
                            wv_sb[:, ct, :],
                            start=(ct == 0), stop=(ct == NCT - 1),
                        )
                for pt in range(NPT):
                    nc.vector.tensor_add(kp_sb[:, pt, :], kp_ps[:, pt, :], eb_sb)
                ones_r = ones[:].rearrange("p (h d) -> p h d", d=64)
                for db in range(NDB):
                    vpa_v = vpa_sb[:, db, :].rearrange(
                        "p (h two d) -> p h two d", two=2, d=64
                    )
                    nc.sync.dma_start(vpa_v[:, :, 0, :], ones_r)
                    nc.vector.tensor_scalar_add(
                        vpa_v[:, :, 1, :],
                        vp_ps[:, db, :].rearrange("p (h d) -> p h d", d=64),
                        fb_sb[:, db:db + 1],
                    )

                # chunks 4-7 are deferred into pass 2 as PE filler (see
                # deferred_qt below); their xt tiles are prefetched here
                for ch in range(NCH - NDEF):
                    xt_t = p1b.tile([128, NCT, SCH], BF16, name="xt")
                    nc.sync.dma_start(
                        xt_t, xT[ch].rearrange("p (ct s) -> p ct s", s=SCH)
                    )
                    dch = NCH - NDEF + ch
                    if dch < NCH:
                        nc.sync.dma_start(
                            xt_d[dch],
                            xT[dch].rearrange("p (ct s) -> p ct s", s=SCH),
                        )
                    for pt in range(NPT):
                        qps = qtps.tile([128, SCH], F32, name="qps")
                        for ct in range(NCT):
                            nc.tensor.matmul(
                                qps,
                                wq_sb[:, ct, pt * 128:(pt + 1) * 128],
                                xt_t[:, ct, :],
                                start=(ct == 0), stop=(ct == NCT - 1),
                            )
                        nc.scalar.copy(qt_sb[ch][:, pt, :], qps)

            # ---------------- pass 2: attention + output projection -------
            # Software pipeline over (chunk, pair) items: scores+exp
            # (stage A) runs 3 items ahead of AV/denominator/normalize
            # (stage B), so the PE never waits on ACT exps; each chunk's
            # output projection lands after the next chunk's first two
            # stage-A emissions, covering the chunk boundary.
            with (
                tc.tile_pool(name="p2wo", bufs=1) as p2wo,
                tc.tile_pool(name="p2ex", bufs=4) as p2ex,
                tc.tile_pool(name="p2ho", bufs=2) as p2ho,
                tc.tile_pool(name="p2rc", bufs=1) as p2rc,
                tc.tile_pool(name="p2out", bufs=2) as p2out,
                tc.tile_pool(name="scps", bufs=3, space="PSUM") as scps,
                # separate pools so AV matmuls never wait on slots held by
                # output-projection copies (and vice versa)
                tc.tile_pool(name="avps", bufs=3, space="PSUM") as avps,
                tc.tile_pool(name="outps", bufs=2, space="PSUM") as outps,
            ):
                wo_sb = p2wo.tile([128, NPT, C], BF16)
                nc.sync.dma_start(
                    wo_sb, woT[:].rearrange("p (pt c) -> p pt c", c=C)
                )
                ho_tiles = {}

                def deferred_qt(ch, pt):
                    # dependency-free PE filler between pass-2 items: keeps
                    # the PE instruction queue primed across the exp/
                    # normalize dependency boundaries (shares avps slots)
                    qps = avps.tile([128, SCH], F32, name="avps")
                    for ct in range(NCT):
                        nc.tensor.matmul(
                            qps,
                            wq_sb[:, ct, pt * 128:(pt + 1) * 128],
                            xt_d[ch][:, ct, :],
                            start=(ct == 0), stop=(ct == NCT - 1),
                        )
                    nc.scalar.copy(qt_sb[ch][:, pt, :], qps)

                def stage_a(ch, pt):
                    qt_c = qt_sb[ch][:, pt, :]
                    ex = p2ex.tile([128, 2, NDB, SCH], BF16, name="ex")
                    for j in range(NDB):
                        # emit the two row-group partners (partition bases
                        # 0/64) back-to-back so the PE merges them into one
                        # concurrent pair; exps follow once both issued
                        scp_j = []
                        for hrow in range(2):
                            lo, hi = hrow * 64, (hrow + 1) * 64
                            scp = scps.tile([128, SCH], F32, name="scp")
                            nc.tensor.matmul(
                                scp,
                                kp_sb[lo:hi, pt, j * 128:(j + 1) * 128],
                                qt_c[lo:hi, :],
                                start=True, stop=True,
                            )
                            scp_j.append((hrow, scp))
                        for hrow, scp in scp_j:
                            nc.scalar.activation(
                                ex[:, hrow, j, :], scp, EXP, scale=0.125
                            )
                    return ex

                def stage_b(ch, pt, ex):
                    # per-head: one matmul group with the augmented
                    # [ones | VpT] stationary operand gives the replicated
                    # denominator (rows 0-63) and unnormalized AV (rows
                    # 64-127) in one PSUM tile; then reciprocal + multiply.
                    if pt == 0:
                        ho_tiles[ch] = p2ho.tile(
                            [128, NPT, SCH], BF16, name="ho_sb"
                        )
                    ho_sb = ho_tiles[ch]
                    avs = []
                    for hrow in range(2):
                        a0 = pt * 256 + hrow * 128
                        av = avps.tile([128, SCH], F32, name="avps")
                        for kt in range(NDB):
                            nc.tensor.matmul(
                                av,
                                vpa_sb[:, kt, a0:a0 + 128],
                                ex[:, hrow, kt, :],
                                start=(kt == 0), stop=(kt == NDB - 1),
                            )
                        avs.append(av)
                    for hrow, av in enumerate(avs):
                        # the custom-DVE reciprocal stays fully at base 0 —
                        # custom ops misread partitions at nonzero bases.
                        rc = p2rc.tile([64, SCH], F32, name="rc")
                        nc.vector.reciprocal_approx_fast(rc, av[0:64, :])
                        lo = hrow * 64
                        nc.vector.tensor_mul(
                            ho_sb[lo:lo + 64, pt, :], av[64:128, :], rc
                        )

                def outproj(ch, half=None):
                    # half-chunk granularity spreads the projection matmuls
                    # across two item boundaries as extra late-phase filler
                    if half == 0:
                        sts = range(0, 2)
                        ho_sb = ho_tiles[ch]
                    elif half == 1:
                        sts = range(2, SCH // 128)
                        ho_sb = ho_tiles.pop(ch)
                    else:
                        sts = range(SCH // 128)
                        ho_sb = ho_tiles.pop(ch)
                    for st in sts:
                        osb = p2out.tile([128, C], BF16, name="osb")
                        row = ch * SCH + st * 128
                        for cc in range(2):
                            ops = outps.tile([128, 512], F32, name="outps")
                            for pt in range(NPT):
                                nc.tensor.matmul(
                                    ops,
                                    ho_sb[:, pt, st * 128:(st + 1) * 128],
                                    wo_sb[:, pt, cc * 512:(cc + 1) * 512],
                                    start=(pt == 0), stop=(pt == NPT - 1),
                                )
                            cs = slice(cc * 512, (cc + 1) * 512)
                            nc.scalar.copy(osb[:, cs], ops)
                        # one full-row DMA (2 KiB contiguous lines) halves
                        # the descriptor count vs per-half DMAs
                        nc.sync.dma_start(out[row:row + 128, :], osb)

                items = [(ch, pt) for ch in range(NCH) for pt in range(NPT)]
                deferred = [
                    (ch, pt)
                    for ch in range(NCH - NDEF, NCH)
                    for pt in range(NPT)
                ]
                DEPTH = 4
                ex_tiles = {}
                for i in range(DEPTH):
                    ex_tiles[items[i]] = stage_a(*items[i])
                for i, (ch, pt) in enumerate(items):
                    if i + DEPTH < len(items):
                        ex_tiles[items[i + DEPTH]] = stage_a(*items[i + DEPTH])
                    stage_b(ch, pt, ex_tiles.pop((ch, pt)))
                    if i < len(deferred):
                        deferred_qt(*deferred[i])
                    # chunk ch's output projection is deferred one item so
                    # the DVE normalize of its last head pair has drained
                    # by the time the PE picks it up, split into halves to
                    # cover two item boundaries
                    if ch > 0 and pt in (0, 1):
                        outproj(ch - 1, half=pt)
                outproj(NCH - 1)

    nc.compile()
    return nc


def get_compiled():
    global _compiled
    with _lock:
        if _compiled is None:
            _compiled = _build()
    return _compiled


def _pack_pm(a, p=128):
    """[N*p, M] -> partition-major [N, p, M] -> host-contiguous [p, N, M]
    flattened per the kernel's expected DMA layout."""
    n = a.shape[0] // p
    return np.ascontiguousarray(
        a.reshape(n, p, a.shape[1]).transpose(1, 0, 2).reshape(p, -1)
    )


def make_in_maps(x, Wq, Wk, Wv, E_w, E_b, F_w, F_b, Wo, bo):
    """Host-side sharding: core i -> (batch i//2, head-group i%2).

    All weight/stream tensors are packed so each kernel DMA reads fat
    contiguous per-partition lines.
    """
    f = np.float32
    bh = BF16_NP
    x = np.asarray(x, f)
    effw = np.concatenate(
        [np.asarray(E_w, f).T, np.asarray(F_w, f).T], axis=1
    ).astype(bh)                                            # [S, 2*DK]
    # st-pair packing: [NST/2, 128, 2*M] with the pair's two 128-row
    # groups side by side in the free axis
    effw_p = np.ascontiguousarray(
        effw.reshape(NST // 2, 2, 128, 2 * DK).transpose(0, 2, 1, 3)
    ).reshape(NST // 2, 128, 4 * DK)
    in_maps = []
    xn_b, xt_b = {}, {}
    for b in range(B):
        xb = x[b].astype(bh)                                # [S, C]
        xn_b[b] = np.ascontiguousarray(
            xb.reshape(NST // 2, 2, 128, C).transpose(0, 2, 1, 3)
        ).reshape(NST // 2, 128, 2 * C)
        xt = np.ascontiguousarray(xb.T)                     # [C, S] bf16
        # [NCH, 128, NCT*SCH]: chunk-major, ct blocks side by side
        xt_b[b] = np.ascontiguousarray(
            xt.reshape(NCT, 128, NCH, SCH).transpose(2, 1, 0, 3)
        ).reshape(NCH, 128, NCT * SCH)
    for core in range(NCORES):
        b, g = divmod(core, 2)
        hs = slice(g * HG, (g + 1) * HG)
        wq = np.asarray(Wq, f)[hs].reshape(HD, C)
        wk = np.asarray(Wk, f)[hs].reshape(HD, C)
        wv = np.asarray(Wv, f)[hs].reshape(HD, C)
        wo = np.asarray(Wo, f)[:, g * HD:(g + 1) * HD]      # [C, 512]
        in_maps.append({
            "xn": xn_b[b],
            "xt": xt_b[b],
            "effw": effw_p,
            "wqt": _pack_pm(np.ascontiguousarray(wq.T).astype(bh)),
            "wkt": _pack_pm(np.ascontiguousarray(wk.T).astype(bh)),
            "wvt": _pack_pm(np.ascontiguousarray(wv.T).astype(bh)),
            "eb": np.asarray(E_b, f),
            "fb": np.asarray(F_b, f),
            "wot": _pack_pm(np.ascontiguousarray(wo.T).astype(bh)),
            "ones": np.ones((128, HD), bh),
        })
    return in_maps


def assemble(results, bo):
    out = np.empty((B, S, C), np.float32)
    for b in range(B):
        out[b] = results[2 * b]["out"].astype(np.float32)
        out[b] += results[2 * b + 1]["out"].astype(np.float32)
    out += np.asarray(bo, np.float32)[None, None, :]
    return out


def kernel(x, Wq, Wk, Wv, E_w, E_b, F_w, F_b, Wo, bo):
    from concourse.bass_utils import run_bass_kernel_spmd

    nc = get_compiled()
    in_maps = make_in_maps(x, Wq, Wk, Wv, E_w, E_b, F_w, F_b, Wo, bo)
    res = run_bass_kernel_spmd(nc, in_maps, core_ids=list(range(NCORES)))
    return assemble(res.results, bo)


# revision 89
# speedup vs baseline: 1.0185x; 1.0185x over previous
"""Linformer-style multi-head attention on 8 Trainium2 NeuronCores.

Problem (hardcoded): B=4, S=4096, C=1024, H=16, D=64, DK=256, fp32 in/out.

Sharding: core i handles (batch b = i//2, head-group g = i%2 of 8 heads).
Each core computes its 8 heads' attention and the partial output
projection out_part = head_out_g @ Wo[:, g_cols].T; the host sums the two
head-group partials per batch and adds bo.

Key algebraic restructuring vs the naive path: the Linformer projections
are reassociated as Kp = (E@x)@Wk^T and Vp = (F@x)@Wv^T, so the sequence
axis (4096) is contracted down to dim_k (256) BEFORE the channel-space
weights are applied.  That removes the full-seq K/V projections entirely
(x@Wk^T and x@Wv^T at [4096,1024]@[1024,512] each) and replaces them with
E@x / F@x at [256,4096]@[4096,1024] plus two tiny [*,1024]@[1024,*]
GEMMs — about 75us less PE time per core.  All matmul operands are bf16
(fp32 PSUM accumulation); measured end-to-end relative error ~7e-3
against the fp32 reference (gate 2e-2).

Per-core kernel:
  pass 1a: stream x (natural [s,c] layout) st-tile by st-tile; one
      matmul per (st, ct) with stationary x-tile and moving [ew|fw]
      tile accumulates EFx^T[c, 2*dk] = [E@x | F@x]^T into 8 PSUM banks
      (one per c-tile) over the full sequence; copy to SBUF bf16.
  pass 1c: Kp^T[hd,dk] = wkT-tiles x ExT-slices; Vp^T[dk,hd] =
      FxT-slices x wvT-tiles (PSUM accum over the 8 c-tiles); add E/F
      biases; build the augmented [ones | Vp^T] stationary vpa.
  pass 1b: Q^T[hd,s] per 512-seq chunk from xT (c-major) and wqT,
      kept resident in SBUF (4 MiB bf16).
  pass 2, software-pipelined over (chunk, head-pair) items (scores+exp
  run 3 items ahead of AV/normalize):
      scoresT[dk,s] = Kp-slices x Q^T   (row-packed head pairs, K=64,
          concurrent via PE row groups at partition bases 0/64)
      expT = exp(scoresT/8) on ACT      (batched [128,1024] per head row)
      per head, ONE matmul group with vpa: rows 0-63 = softmax
          denominator replicated across partitions, rows 64-127 =
          unnormalized head_out^T; then ho = av * recip_approx(denom)
          on DVE (cross-partition-base operands, probed valid on HW)
      out[s,c] = ho-slices x WoT        (accumulate 4 hd blocks; PSUM
          slots shared with the AV pool; copies on ACT)
"""

import threading

import ml_dtypes
import numpy as np

B, S, C = 4, 4096, 1024
H, D, DK = 16, 64, 256
HG = 8               # heads per core
HD = HG * D          # 512
NCORES = 8
SCH = 512            # sequence chunk (pass 1b / pass 2)
NCH = S // SCH       # 8 chunks
NST = S // 128       # 32 s-tiles over the full sequence
NCT = C // 128       # 8 c-tiles
NPT = HD // 128      # 4 hd blocks (head pairs)
NDB = DK // 128      # 2 dk blocks

BF16_NP = ml_dtypes.bfloat16

_lock = threading.Lock()
_compiled = None


def _build():
    import concourse.bacc as bacc
    import concourse.bass as bass
    import concourse.tile as tile
    from concourse import mybir

    F32 = mybir.dt.float32
    BF16 = mybir.dt.bfloat16
    EXP = mybir.ActivationFunctionType.Exp

    nc = bacc.Bacc(None, target_bir_lowering=False)

    # all inputs are host-packed partition-major so every DMA moves fat
    # contiguous per-partition lines (2-8 KiB)
    xn = nc.dram_tensor("xn", [NST // 2, 128, 2 * C], BF16, kind="ExternalInput")
    xT = nc.dram_tensor("xt", [NCH, 128, NCT * SCH], BF16, kind="ExternalInput")
    effw = nc.dram_tensor(
        "effw", [NST // 2, 128, 4 * DK], BF16, kind="ExternalInput"
    )
    wqT = nc.dram_tensor("wqt", [128, NCT * HD], BF16, kind="ExternalInput")
    wkT = nc.dram_tensor("wkt", [128, NCT * HD], BF16, kind="ExternalInput")
    wvT = nc.dram_tensor("wvt", [128, NCT * HD], BF16, kind="ExternalInput")
    eb = nc.dram_tensor("eb", [DK], F32, kind="ExternalInput")
    fb = nc.dram_tensor("fb", [DK], F32, kind="ExternalInput")
    woT = nc.dram_tensor("wot", [128, NPT * C], BF16, kind="ExternalInput")
    ones = nc.dram_tensor("ones", [128, HD], BF16, kind="ExternalInput")
    out = nc.dram_tensor("out", [S, C], BF16, kind="ExternalOutput")

    with tile.TileContext(nc) as tc:
        with (
            tc.tile_pool(name="consts", bufs=1) as consts,
            tc.tile_pool(name="mids", bufs=1) as mids,
        ):
            wq_sb = consts.tile([128, NCT, HD], BF16)
            wk_sb = consts.tile([128, NCT, HD], BF16)
            wv_sb = consts.tile([128, NCT, HD], BF16)
            eb_sb = consts.tile([128, DK], F32)
            fb_sb = consts.tile([128, NDB], F32)

            # per-ct tiles so 1c matmuls only depend on their own copy
            efx_sb = [
                mids.tile([128, 2 * DK], BF16, name=f"efxsb{ct}")
                for ct in range(NCT)
            ]
            kp_sb = mids.tile([128, NPT, DK], BF16)         # Kp^T [hd, dk]
            # Augmented Vp^T: per dk-tile, per head: 64 cols of ones then
            # 64 cols of Vp^T. A single AV matmul then yields rows 0-63 =
            # the softmax denominator replicated across 64 partitions and
            # rows 64-127 = head_out^T (unnormalized).
            vpa_sb = mids.tile([128, NDB, 2 * HD], BF16)
            # Q^T, per-chunk tiles so pass 2's first scores only depend on
            # chunk 0's copies, not all 32
            qt_sb = [
                mids.tile([128, NPT, SCH], BF16, name=f"qt{ch}")
                for ch in range(NCH)
            ]
            # xt for the deferred Q chunks (long-lived: written in 1b,
            # read by pass-2 filler matmuls)
            NDEF = 4
            xt_d = {
                ch: mids.tile([128, NCT, SCH], BF16, name=f"xtd{ch}")
                for ch in range(NCH - NDEF, NCH)
            }

            # ------- pass 1a: EFx^T = [E@x | F@x]^T, full-seq accum -------
            with (
                tc.tile_pool(name="p1a", bufs=5) as p1a,
                tc.tile_pool(name="efxps", bufs=1, space="PSUM") as efxps,
            ):
                # one PSUM tile (= one bank) per c-tile so the trailing
                # copies only depend on their own bank's stop matmul
                efx_ps = [
                    efxps.tile([128, 2 * DK], F32, name=f"efx{ct}")
                    for ct in range(NCT)
                ]
                for sp in range(NST // 2):
                    xn_t = p1a.tile([128, 2, C], BF16, name="xn")
                    ef_t = p1a.tile([128, 2, 2 * DK], BF16, name="ef")
                    if sp == 0:
                        # split the first tiles into halves so the first
                        # matmul starts as early as possible
                        for half in range(2):
                            nc.sync.dma_start(ef_t[:, half, :], effw[sp][:, half * 2 * DK:(half + 1) * 2 * DK])
                            nc.sync.dma_start(xn_t[:, half, :], xn[sp][:, half * C:(half + 1) * C])
                    else:
                        nc.sync.dma_start(ef_t, effw[sp])
                        nc.sync.dma_start(xn_t, xn[sp])
                    # consts for 1c/1b, spread so no burst stalls the stream
                    if sp == 2:
                        nc.sync.dma_start(
                            wk_sb, wkT[:].rearrange("p (ct n) -> p ct n", n=HD)
                        )
                    elif sp == 6:
                        nc.sync.dma_start(
                            wv_sb, wvT[:].rearrange("p (ct n) -> p ct n", n=HD)
                        )
                    elif sp == 10:
                        nc.sync.dma_start(
                            wq_sb, wqT[:].rearrange("p (ct n) -> p ct n", n=HD)
                        )
                    elif sp == 14:
                        eb_bc = bass.AP(
                            tensor=eb[:].tensor, offset=0, ap=[[0, 128], [1, DK]]
                        )
                        nc.sync.dma_start(eb_sb[:], eb_bc)
                        for db in range(NDB):
                            fb_col = fb[db * 128:(db + 1) * 128].rearrange(
                                "(p one) -> p one", one=1
                            )
                            nc.sync.dma_start(fb_sb[:, db:db + 1], fb_col)
                    for half in range(2):
                        st = 2 * sp + half
                        for ct in range(NCT):
                            nc.tensor.matmul(
                                efx_ps[ct],
                                xn_t[:, half, ct * 128:(ct + 1) * 128],
                                ef_t[:, half, :],
                                start=(st == 0), stop=(st == NST - 1),
                            )
                            if st == NST - 1:
                                # alternate engines: copies trail the last
                                # matmuls and gate 1c
                                if ct % 2 == 0:
                                    nc.scalar.copy(efx_sb[ct], efx_ps[ct])
                                else:
                                    nc.vector.tensor_copy(efx_sb[ct], efx_ps[ct])

            # ------- pass 1c: Kp^T / Vp^T (tiny) + pass 1b: Q^T ----------
            with (
                tc.tile_pool(name="kvps", bufs=1, space="PSUM") as kvps,
                tc.tile_pool(name="qtps", bufs=2, space="PSUM") as qtps,
                tc.tile_pool(name="p1b", bufs=2) as p1b,
            ):
                kp_ps = kvps.tile([128, NPT, DK], F32)
                vp_ps = kvps.tile([128, NDB, HD], F32)
                # ct-outer so the PE starts as soon as efx_sb[ct=0] lands;
                # kp_ps slices pt={0,1} share PSUM bank 0 and pt={2,3}
                # share bank 1 — one accumulation group per bank.
                for ct in range(NCT):
                    for pt in range(NPT):
                        nc.tensor.matmul(
                            kp_ps[:, pt, :],
                            wk_sb[:, ct, pt * 128:(pt + 1) * 128],
                            efx_sb[ct][:, 0:DK],
                            start=(pt % 2 == 0 and ct == 0),
                            stop=(pt % 2 == 1 and ct == NCT - 1),
                        )
                    for db in range(NDB):
                        nc.tensor.matmul(
                            vp_ps[:, db, :],
                            efx_sb[ct][:, DK + db * 128:DK + (db + 1) * 128],
                            wv_sb[:, ct, :],
                            start=(ct == 0), stop=(ct == NCT - 1),
                        )
                for pt in range(NPT):
                    nc.vector.tensor_add(kp_sb[:, pt, :], kp_ps[:, pt, :], eb_sb)
                ones_r = ones[:].rearrange("p (h d) -> p h d", d=64)
                for db in range(NDB):
                    vpa_v = vpa_sb[:, db, :].rearrange(
                        "p (h two d) -> p h two d", two=2, d=64
                    )
                    nc.sync.dma_start(vpa_v[:, :, 0, :], ones_r)
                    nc.vector.tensor_scalar_add(
                        vpa_v[:, :, 1, :],
                        vp_ps[:, db, :].rearrange("p (h d) -> p h d", d=64),
                        fb_sb[:, db:db + 1],
                    )

                # chunks 4-7 are deferred into pass 2 as PE filler (see
                # deferred_qt below); their xt tiles are prefetched here
                for ch in range(NCH - NDEF):
                    xt_t = p1b.tile([128, NCT, SCH], BF16, name="xt")
                    nc.sync.dma_start(
                        xt_t, xT[ch].rearrange("p (ct s) -> p ct s", s=SCH)
                    )
                    dch = NCH - NDEF + ch
                    if dch < NCH:
                        nc.sync.dma_start(
                            xt_d[dch],
                            xT[dch].rearrange("p (ct s) -> p ct s", s=SCH),
                        )
                    for pt in range(NPT):
                        qps = qtps.tile([128, SCH], F32, name="qps")
                        for ct in range(NCT):
                            nc.tensor.matmul(
                                qps,
                                wq_sb[:, ct, pt * 128:(pt + 1) * 128],
                                xt_t[:, ct, :],
                                start=(ct == 0), stop=(ct == NCT - 1),
                            )
                        nc.scalar.copy(qt_sb[ch][:, pt, :], qps)

            # ---------------- pass 2: attention + output projection -------
            # Software pipeline over (chunk, pair) items: scores+exp
            # (stage A) runs 3 items ahead of AV/denominator/normalize
            # (stage B), so the PE never waits on ACT exps; each chunk's
            # output projection lands after the next chunk's first two
            # stage-A emissions, covering the chunk boundary.
            with (
                tc.tile_pool(name="p2wo", bufs=1) as p2wo,
                tc.tile_pool(name="p2ex", bufs=4) as p2ex,
                tc.tile_pool(name="p2ho", bufs=2) as p2ho,
                tc.tile_pool(name="p2rc", bufs=1) as p2rc,
                tc.tile_pool(name="p2out", bufs=2) as p2out,
                tc.tile_pool(name="scps", bufs=3, space="PSUM") as scps,
                # separate pools so AV matmuls never wait on slots held by
                # output-projection copies (and vice versa)
                tc.tile_pool(name="avps", bufs=3, space="PSUM") as avps,
                tc.tile_pool(name="outps", bufs=2, space="PSUM") as outps,
            ):
                wo_sb = p2wo.tile([128, NPT, C], BF16)
                nc.sync.dma_start(
                    wo_sb, woT[:].rearrange("p (pt c) -> p pt c", c=C)
                )
                ho_tiles = {}

                def deferred_qt(ch, pt):
                    # dependency-free PE filler between pass-2 items: keeps
                    # the PE instruction queue primed across the exp/
                    # normalize dependency boundaries (shares avps slots)
                    qps = avps.tile([128, SCH], F32, name="avps")
                    for ct in range(NCT):
                        nc.tensor.matmul(
                            qps,
                            wq_sb[:, ct, pt * 128:(pt + 1) * 128],
                            xt_d[ch][:, ct, :],
                            start=(ct == 0), stop=(ct == NCT - 1),
                        )
                    nc.scalar.copy(qt_sb[ch][:, pt, :], qps)

                def stage_a(ch, pt):
                    qt_c = qt_sb[ch][:, pt, :]
                    ex = p2ex.tile([128, 2, NDB, SCH], BF16, name="ex")
                    for j in range(NDB):
                        # emit the two row-group partners (partition bases
                        # 0/64) back-to-back so the PE merges them into one
                        # concurrent pair; exps follow once both issued
                        scp_j = []
                        for hrow in range(2):
                            lo, hi = hrow * 64, (hrow + 1) * 64
                            scp = scps.tile([128, SCH], F32, name="scp")
                            nc.tensor.matmul(
                                scp,
                                kp_sb[lo:hi, pt, j * 128:(j + 1) * 128],
                                qt_c[lo:hi, :],
                                start=True, stop=True,
                            )
                            scp_j.append((hrow, scp))
                        for hrow, scp in scp_j:
                            nc.scalar.activation(
                                ex[:, hrow, j, :], scp, EXP, scale=0.125
                            )
                    return ex

                def stage_b(ch, pt, ex):
                    # per-head: one matmul group with the augmented
                    # [ones | VpT] stationary operand gives the replicated
                    # denominator (rows 0-63) and unnormalized AV (rows
                    # 64-127) in one PSUM tile; then reciprocal + multiply.
                    if pt == 0:
                        ho_tiles[ch] = p2ho.tile(
                            [128, NPT, SCH], BF16, name="ho_sb"
                        )
                    ho_sb = ho_tiles[ch]
                    avs = []
                    for hrow in range(2):
                        a0 = pt * 256 + hrow * 128
                        av = avps.tile([128, SCH], F32, name="avps")
                        for kt in range(NDB):
                            nc.tensor.matmul(
                                av,
                                vpa_sb[:, kt, a0:a0 + 128],
                                ex[:, hrow, kt, :],
                                start=(kt == 0), stop=(kt == NDB - 1),
                            )
                        avs.append(av)
                    for hrow, av in enumerate(avs):
                        # the custom-DVE reciprocal stays fully at base 0 —
                        # custom ops misread partitions at nonzero bases.
                        rc = p2rc.tile([64, SCH], F32, name="rc")
                        nc.vector.reciprocal_approx_fast(rc, av[0:64, :])
                        lo = hrow * 64
                        nc.vector.tensor_mul(
                            ho_sb[lo:lo + 64, pt, :], av[64:128, :], rc
                        )

                def outproj(ch, half=None):
                    # half-chunk granularity spreads the projection matmuls
                    # across two item boundaries as extra late-phase filler
                    if half == 0:
                        sts = range(0, 2)
                        ho_sb = ho_tiles[ch]
                    elif half == 1:
                        sts = range(2, SCH // 128)
                        ho_sb = ho_tiles.pop(ch)
                    else:
                        sts = range(SCH // 128)
                        ho_sb = ho_tiles.pop(ch)
                    for st in sts:
                        osb = p2out.tile([128, C], BF16, name="osb")
                        row = ch * SCH + st * 128
                        for cc in range(2):
                            ops = outps.tile([128, 512], F32, name="outps")
                            for pt in range(NPT):
                                nc.tensor.matmul(
                                    ops,
                                    ho_sb[:, pt, st * 128:(st + 1) * 128],
                                    wo_sb[:, pt, cc * 512:(cc + 1) * 512],
                                    start=(pt == 0), stop=(pt == NPT - 1),
                                )
                            cs = slice(cc * 512, (cc + 1) * 512)
                            nc.scalar.copy(osb[:, cs], ops)
                        # one full-row DMA (2 KiB contiguous lines) halves
                        # the descriptor count vs per-half DMAs
                        nc.sync.dma_start(out[row:row + 128, :], osb)

                items = [(ch, pt) for ch in range(NCH) for pt in range(NPT)]
                deferred = [
                    (ch, pt)
                    for ch in range(NCH - NDEF, NCH)
                    for pt in range(NPT)
                ]
                DEPTH = 4
                ex_tiles = {}
                for i in range(DEPTH):
                    ex_tiles[items[i]] = stage_a(*items[i])
                for i, (ch, pt) in enumerate(items):
                    if i + DEPTH < len(items):
                        ex_tiles[items[i + DEPTH]] = stage_a(*items[i + DEPTH])
                    stage_b(ch, pt, ex_tiles.pop((ch, pt)))
                    if i < len(deferred):
                        deferred_qt(*deferred[i])
                    # chunk ch's output projection is deferred one item so
                    # the DVE normalize of its last head pair has drained
                    # by the time the PE picks it up, split into halves to
                    # cover two item boundaries
                    if ch > 0 and pt in (0, 1):
                        outproj(ch - 1, half=pt)
                outproj(NCH - 1)

    nc.compile()
    return nc


def get_compiled():
    global _compiled
    with _lock:
        if _compiled is None:
            _compiled = _build()
    return _compiled


def _pack_pm(a, p=128):
    """[N*p, M] -> partition-major [N, p, M] -> host-contiguous [p, N, M]
    flattened per the kernel's expected DMA layout."""
    n = a.shape[0] // p
    return np.ascontiguousarray(
        a.reshape(n, p, a.shape[1]).transpose(1, 0, 2).reshape(p, -1)
    )


def make_in_maps(x, Wq, Wk, Wv, E_w, E_b, F_w, F_b, Wo, bo):
    """Host-side sharding: core i -> (batch i//2, head-group i%2).

    All weight/stream tensors are packed so each kernel DMA reads fat
    contiguous per-partition lines.
    """
    f = np.float32
    bh = BF16_NP
    x = np.asarray(x, f)
    effw = np.concatenate(
        [np.asarray(E_w, f).T, np.asarray(F_w, f).T], axis=1
    ).astype(bh)                                            # [S, 2*DK]
    # st-pair packing: [NST/2, 128, 2*M] with the pair's two 128-row
    # groups side by side in the free axis
    effw_p = np.ascontiguousarray(
        effw.reshape(NST // 2, 2, 128, 2 * DK).transpose(0, 2, 1, 3)
    ).reshape(NST // 2, 128, 4 * DK)
    in_maps = []
    xn_b, xt_b = {}, {}
    for b in range(B):
        xb = x[b].astype(bh)                                # [S, C]
        xn_b[b] = np.ascontiguousarray(
            xb.reshape(NST // 2, 2, 128, C).transpose(0, 2, 1, 3)
        ).reshape(NST // 2, 128, 2 * C)
        xt = np.ascontiguousarray(xb.T)                     # [C, S] bf16
        # [NCH, 128, NCT*SCH]: chunk-major, ct blocks side by side
        xt_b[b] = np.ascontiguousarray(
            xt.reshape(NCT, 128, NCH, SCH).transpose(2, 1, 0, 3)
        ).reshape(NCH, 128, NCT * SCH)
    for core in range(NCORES):
        b, g = divmod(core, 2)
        hs = slice(g * HG, (g + 1) * HG)
        wq = np.asarray(Wq, f)[hs].reshape(HD, C)
        wk = np.asarray(Wk, f)[hs].reshape(HD, C)
        wv = np.asarray(Wv, f)[hs].reshape(HD, C)
        wo = np.asarray(Wo, f)[:, g * HD:(g + 1) * HD]      # [C, 512]
        in_maps.append({
            "xn": xn_b[b],
            "xt": xt_b[b],
            "effw": effw_p,
            "wqt": _pack_pm(np.ascontiguousarray(wq.T).astype(bh)),
            "wkt": _pack_pm(np.ascontiguousarray(wk.T).astype(bh)),
            "wvt": _pack_pm(np.ascontiguousarray(wv.T).astype(bh)),
            "eb": np.asarray(E_b, f),
            "fb": np.asarray(F_b, f),
            "wot": _pack_pm(np.ascontiguousarray(wo.T).astype(bh)),
            "ones": np.ones((128, HD), bh),
        })
    return in_maps


def assemble(results, bo):
    out = np.empty((B, S, C), np.float32)
    for b in range(B):
        out[b] = results[2 * b]["out"].astype(np.float32)
        out[b] += results[2 * b + 1]["out"].astype(np.float32)
    out += np.asarray(bo, np.float32)[None, None, :]
    return out


def kernel(x, Wq, Wk, Wv, E_w, E_b, F_w, F_b, Wo, bo):
    from concourse.bass_utils import run_bass_kernel_spmd

    nc = get_compiled()
    in_maps = make_in_maps(x, Wq, Wk, Wv, E_w, E_b, F_w, F_b, Wo, bo)
    res = run_bass_kernel_spmd(nc, in_maps, core_ids=list(range(NCORES)))
    return assemble(res.results, bo)
